# revision 5
# baseline (speedup 1.0000x reference)
"""Neighborhood attention (NATTEN 7x7) Trainium2 kernel.

Problem: x [4, 1024, 768] -> qkv proj -> 7x7 neighborhood attention on a
32x32 grid with 12 heads -> output proj.  Full inputs in, full output out.

Sharding: data-parallel over (batch, grid-half) = 8 shards.  Each core owns
16 grid rows (512 tokens) and receives a 3-row halo, i.e. 19 rows = 608
tokens.  The bottom half is flipped vertically on the host so that all 8
cores run an identical program (NATTEN clamped windows are reflection
symmetric); the output of flipped shards is un-flipped on the host.

Per-core pipeline (all feature-major / "transposed" layouts):
  1. qT/kT = W_{q,k} @ x^T   [feature-part, token-free]  (f32r matmuls)
  2. v     = x @ W_v^T       [token-part, feature-free], stored with a ones
     column per head (65-stride blocks) so the AV matmul also accumulates
     the softmax denominator.
  3. Key-stationary attention: key tiles of 4 grid rows (128 tokens);
     scores^T [keys, queries] via PE; exp on ACT; mask-mul on DVE with
     host-precomputed NATTEN masks; AV accumulates out^T[hd+1, 512] in PSUM
     across key tiles (no max-subtraction needed: |scores| is O(5)).
  4. Normalize via reciprocal + rank-1 broadcast matmul, then output proj.
"""

import sys

sys.path.insert(0, "/opt/trn_rl_repo")

from contextlib import ExitStack

import numpy as np

import concourse.bacc as bacc
import concourse.mybir as mybir
from concourse import tile
from concourse.bass_utils import run_bass_kernel_spmd

F32 = mybir.dt.float32
F32R = mybir.dt.float32r

B, HG, WG, D, NH, KW = 4, 32, 32, 768, 12, 7
HD = D // NH  # 64
N = HG * WG  # 1024

# Shard geometry (identical for every core; bottom halves are row-flipped).
OWN_ROWS = 16          # grid rows owned per core
HALO = 3               # extra key/value rows
SH_ROWS = OWN_ROWS + HALO      # 19
SH_TOK = SH_ROWS * WG          # 608
OWN_TOK = OWN_ROWS * WG        # 512
KT_ROWS = 4                    # grid rows per key tile
NKT = 5                        # key tiles (last covers 3 rows + 1 pad row)
KPAD = NKT * KT_ROWS * WG      # 640 padded key columns
NQ = 352                       # query window width per key tile (11 rows)
TCH = 304                      # token chunk for qT/kT matmuls (2 x 304 = 608)


def _query_windows():
    """Per key tile: query-token window start (in owned-token coords)."""
    si = np.clip(np.arange(HG) - (KW // 2), 0, HG - KW)
    qlo = []
    for kt in range(NKT):
        kr0, kr1 = kt * KT_ROWS, min(kt * KT_ROWS + KT_ROWS - 1, SH_ROWS - 1)
        qr = [q for q in range(OWN_ROWS) if si[q] <= kr1 and si[q] + KW - 1 >= kr0]
        lo, hi = min(qr), max(qr)
        assert (hi - lo + 1) * WG <= NQ, (kt, lo, hi)
        start = min(lo * WG, OWN_TOK - NQ)
        assert hi * WG + WG <= start + NQ
        qlo.append(start)
    return qlo


QLO = _query_windows()
KL = [min(128, SH_TOK - 128 * k) for k in range(NKT)]  # real keys per tile


def _masks():
    """masks[kt, key_idx, query_idx] in {0,1}: NATTEN neighborhood test."""
    si = np.clip(np.arange(HG) - (KW // 2), 0, HG - KW)
    m = np.zeros((NKT, 128, NQ), dtype=np.float32)
    for kt in range(NKT):
        kk = kt * 128 + np.arange(128)
        kr, kc = kk // WG, kk % WG
        q = QLO[kt] + np.arange(NQ)
        qr, qc = q // WG, q % WG
        row_ok = (si[qr][None, :] <= kr[:, None]) & (kr[:, None] <= si[qr][None, :] + KW - 1)
        col_ok = (si[qc][None, :] <= kc[:, None]) & (kc[:, None] <= si[qc][None, :] + KW - 1)
        valid = (kr < SH_ROWS)[:, None]
        m[kt] = (row_ok & col_ok & valid).astype(np.float32)
    return m


def build_bass():
    nc = bacc.Bacc()
    xT = nc.declare_dram_parameter("xT", [D, SH_TOK], F32R, isOutput=False)
    wT = nc.declare_dram_parameter("wT", [D, 3 * D], F32R, isOutput=False)
    pwT = nc.declare_dram_parameter("pwT", [D, D], F32R, isOutput=False)
    qkvb = nc.declare_dram_parameter("qkvb", [1, 3 * D], F32R, isOutput=False)
    pb = nc.declare_dram_parameter("pb", [1, D], F32R, isOutput=False)
    masks = nc.declare_dram_parameter("masks", [NKT, 128, NQ], F32R, isOutput=False)
    ones = nc.declare_dram_parameter("ones", [1, KPAD], F32R, isOutput=False)
    z65 = nc.declare_dram_parameter("z65", [1, 65], F32R, isOutput=False)
    vinit = nc.declare_dram_parameter("vinit", [128, NH * 65], F32R, isOutput=False)
    out = nc.declare_dram_parameter("out", [OWN_TOK, D], F32, isOutput=True)

    with ExitStack() as ctx:
        tc = ctx.enter_context(tile.TileContext(nc))
        pp = ctx.enter_context(tc.tile_pool(name="persist", bufs=1))
        sc_pool = ctx.enter_context(tc.tile_pool(name="scexp", bufs=3))
        me_pool = ctx.enter_context(tc.tile_pool(name="mexp", bufs=3))
        bc_pool = ctx.enter_context(tc.tile_pool(name="bcast", bufs=2))
        rc_pool = ctx.enter_context(tc.tile_pool(name="recip", bufs=2))
        ob_pool = ctx.enter_context(tc.tile_pool(name="outsb", bufs=2))
        ps_mm = ctx.enter_context(tc.tile_pool(name="psmm", bufs=2, space="PSUM"))
        ps_sc = ctx.enter_context(tc.tile_pool(name="pssc", bufs=2, space="PSUM"))
        ps_att = ctx.enter_context(tc.tile_pool(name="psatt", bufs=2, space="PSUM"))
        ps_bc = ctx.enter_context(tc.tile_pool(name="psbc", bufs=1, space="PSUM"))

        # ---- persistent SBUF tiles + loads ----
        xt = [pp.tile([128, SH_TOK], F32R, tag=f"xt{i}", name=f"xt{i}") for i in range(6)]
        wt = [pp.tile([128, 3 * D], F32R, tag=f"w{i}", name=f"w{i}") for i in range(6)]
        pwt = [pp.tile([128, D], F32R, tag=f"pw{i}", name=f"pw{i}") for i in range(6)]
        qk = [pp.tile([128, SH_TOK], F32R, tag=f"qk{i}", name=f"qk{i}") for i in range(12)]
        vt = [pp.tile([128, NH * 65], F32R, tag=f"v{i}", name=f"v{i}") for i in range(NKT)]
        mt = [pp.tile([128, NQ], F32R, tag=f"m{i}", name=f"m{i}") for i in range(NKT)]
        at = [pp.tile([128, OWN_TOK], F32R, tag=f"at{i}", name=f"at{i}") for i in range(6)]
        qkvb_t = pp.tile([1, 3 * D], F32R, tag="qkvb")
        pb_t = pp.tile([1, D], F32R, tag="pb")
        ones_t = pp.tile([1, KPAD], F32R, tag="ones")
        z65_t = pp.tile([1, 65], F32R, tag="z65")

        for i in range(6):
            nc.sync.dma_start(xt[i][:], xT[128 * i : 128 * i + 128, :])
            nc.sync.dma_start(wt[i][:], wT[128 * i : 128 * i + 128, :])
            nc.sync.dma_start(pwt[i][:], pwT[128 * i : 128 * i + 128, :])
        for k in range(NKT):
            nc.sync.dma_start(mt[k][:], masks[k, :, :])
        nc.sync.dma_start(qkvb_t[:], qkvb[:])
        nc.sync.dma_start(pb_t[:], pb[:])
        nc.sync.dma_start(ones_t[:], ones[:])
        nc.sync.dma_start(z65_t[:], z65[:])

        # v tiles: ones column per head (softmax denominator), zeros elsewhere;
        # evictions later overwrite the 64-wide data blocks.
        for k in range(NKT):
            nc.sync.dma_start(vt[k][:], vinit[:])

        # ---- phase 1a: qT / kT  (feature-major) ----
        for oc in range(12):  # 6 q chunks then 6 k chunks (qkv features 0..1535)
            for th in range(2):
                ps = ps_mm.tile([128, 384], F32, tag="psmm", name="psmm")
                tsl = slice(th * TCH, th * TCH + TCH)
                for d in range(6):
                    nc.tensor.matmul(
                        ps[:, 0:TCH],
                        wt[d][:, 128 * oc : 128 * oc + 128],
                        xt[d][:, tsl],
                        start=(d == 0),
                        stop=False,
                    )
                nc.tensor.matmul(
                    ps[:, 0:TCH],
                    qkvb_t[0:1, 128 * oc : 128 * oc + 128],
                    ones_t[0:1, 0:TCH],
                    start=False,
                    stop=True,
                )
                scale = (HD ** -0.5) if oc < 6 else 1.0
                nc.scalar.activation(
                    qk[oc][:, tsl], ps[:, 0:TCH],
                    mybir.ActivationFunctionType.Copy, scale=scale,
                )

        # ---- phase 1b: v  (token-major, 65-stride head blocks) ----
        for tc5 in range(NKT):
            tl = min(128, SH_TOK - 128 * tc5)  # 128,128,128,128,96
            for oh in range(2):
                ps = ps_mm.tile([128, 384], F32, tag="psmm", name="psmm")
                vcol = 3 * D - D + 384 * oh  # 1536 + 384*oh
                for d in range(6):
                    nc.tensor.matmul(
                        ps[0:tl, :],
                        xt[d][:, 128 * tc5 : 128 * tc5 + tl],
                        wt[d][:, vcol : vcol + 384],
                        start=(d == 0),
                        stop=False,
                    )
                nc.tensor.matmul(
                    ps[0:tl, :],
                    ones_t[0:1, 0:tl],
                    qkvb_t[0:1, vcol : vcol + 384],
                    start=False,
                    stop=True,
                )
                dest = vt[tc5][0:tl, 390 * oh : 390 * oh + 390].rearrange(
                    "p (h c) -> p h c", c=65
                )[:, :, 0:64]
                nc.scalar.copy(dest, ps[0:tl, :])

        # ---- phase 2: attention, head by head ----
        for h in range(NH):
            qt, qrow = qk[h // 2], (h % 2) * 64
            kt_, krow = qk[6 + h // 2], (h % 2) * 64
            po = ps_att.tile([65, OWN_TOK], F32, tag="psatt", name="psatt")
            nc.tensor.matmul(
                po[:], z65_t[0:1, 0:65], ones_t[0:1, 0:OWN_TOK],
                start=True, stop=False,
            )
            for k in range(NKT):
                kl = KL[k]
                ps = ps_sc.tile([128, NQ], F32, tag="pssc", name="pssc")
                nc.tensor.matmul(
                    ps[0:kl, :],
                    kt_[krow : krow + 64, 128 * k : 128 * k + kl],
                    qt[qrow : qrow + 64, QLO[k] : QLO[k] + NQ],
                    start=True,
                    stop=True,
                )
                se = sc_pool.tile([128, NQ], F32R, tag="scexp", name="scexp")
                nc.scalar.activation(se[0:kl, :], ps[0:kl, :], mybir.ActivationFunctionType.Exp)
                me = me_pool.tile([128, NQ], F32R, tag="mexp", name="mexp")
                nc.vector.tensor_mul(me[0:kl, :], se[0:kl, :], mt[k][0:kl, :])
                nc.tensor.matmul(
                    po[:, QLO[k] : QLO[k] + NQ],
                    vt[k][0:kl, 65 * h : 65 * h + 65],
                    me[0:kl, :],
                    start=False,
                    stop=(k == NKT - 1),
                )
            rc = rc_pool.tile([1, OWN_TOK], F32R, tag="recip", name="recip")
            with nc.allow_low_precision(reason="f32r recip for rank-1 bcast"):
                nc.vector.reciprocal(rc[:], po[64:65, :])
            pbc = ps_bc.tile([64, OWN_TOK], F32, tag="psbc", name="psbc")
            nc.tensor.matmul(pbc[:], ones_t[0:1, 0:64], rc[:], start=True, stop=True)
            bcs = bc_pool.tile([64, OWN_TOK], F32, tag="bcast", name="bcast")
            nc.scalar.copy(bcs[:], pbc[:])
            nc.vector.tensor_mul(
                at[h // 2][(h % 2) * 64 : (h % 2) * 64 + 64, :], po[0:64, :], bcs[:]
            )

        # ---- phase 3: output projection ----
        for tc4 in range(4):
            for oh in range(2):
                ps = ps_mm.tile([128, 384], F32, tag="psmm", name="psmm")
                for d in range(6):
                    nc.tensor.matmul(
                        ps[:],
                        at[d][:, 128 * tc4 : 128 * tc4 + 128],
                        pwt[d][:, 384 * oh : 384 * oh + 384],
                        start=(d == 0),
                        stop=False,
                    )
                nc.tensor.matmul(
                    ps[:],
                    ones_t[0:1, 0:128],
                    pb_t[0:1, 384 * oh : 384 * oh + 384],
                    start=False,
                    stop=True,
                )
                o = ob_pool.tile([128, 384], F32, tag="outsb", name="outsb")
                nc.scalar.copy(o[:], ps[:])
                nc.sync.dma_start(
                    out[128 * tc4 : 128 * tc4 + 128, 384 * oh : 384 * oh + 384], o[:]
                )
    nc.compile()
    return nc


_CACHE = {}


def _get_nc():
    if "nc" not in _CACHE:
        _CACHE["nc"] = build_bass()
    return _CACHE["nc"]


def kernel(x, qkv_w, qkv_b, proj_w, proj_b):
    x = np.asarray(x, dtype=np.float32)
    qkv_w = np.asarray(qkv_w, dtype=np.float32)
    qkv_b = np.asarray(qkv_b, dtype=np.float32)
    proj_w = np.asarray(proj_w, dtype=np.float32)
    proj_b = np.asarray(proj_b, dtype=np.float32)

    nc = _get_nc()

    wTn = np.ascontiguousarray(qkv_w.T)              # [768, 2304]
    pwTn = np.ascontiguousarray(proj_w.T)            # [768, 768]
    masks_n = _masks()
    ones_n = np.ones((1, KPAD), dtype=np.float32)
    z65_n = np.zeros((1, 65), dtype=np.float32)
    vinit_n = np.zeros((128, NH * 65), dtype=np.float32)
    vinit_n[:, 64::65] = 1.0
    qkvb_n = qkv_b.reshape(1, 3 * D)
    pb_n = proj_b.reshape(1, D)

    cols = np.arange(WG)
    shard_rows = []   # local row -> grid row, per half
    for hh in range(2):
        rows = np.arange(SH_ROWS) if hh == 0 else (HG - 1 - np.arange(SH_ROWS))
        shard_rows.append(rows)

    in_maps = []
    for c in range(8):
        b, hh = c // 2, c % 2
        rows = shard_rows[hh]
        toks = (rows[:, None] * WG + cols[None, :]).reshape(-1)
        xTn = np.ascontiguousarray(x[b, toks, :].T)  # [768, 608]
        in_maps.append(
            dict(xT=xTn, wT=wTn, pwT=pwTn, qkvb=qkvb_n, pb=pb_n,
                 masks=masks_n, ones=ones_n, z65=z65_n, vinit=vinit_n)
        )

    res = run_bass_kernel_spmd(nc, in_maps, list(range(8)))

    full = np.empty((B, N, D), dtype=np.float32)
    for c in range(8):
        b, hh = c // 2, c % 2
        own_rows = shard_rows[hh][:OWN_ROWS]
        toks = (own_rows[:, None] * WG + cols[None, :]).reshape(-1)
        full[b, toks, :] = res.results[c]["out"]
    return full
